# revision 6
# baseline (speedup 1.0000x reference)
"""Trainium2 Bass kernel v9 for nn_BilinearMixture.

Design (measurement-driven):
- Edges u-sorted, block-partitioned over 8 cores; cut into segments of
  <=2048 edges / <=128 distinct users with per-segment class quotas
  QAS=QBS=1024 (VSPLIT=25088 makes the v-table halves equiprobable, so
  all four per-tile dma_gather calls are EQUAL 1024-idx calls ->
  queue-parallel; unequal calls measured 2x slower).
- v side: 4 plain dma_gather calls/tile into SEPARATE dst tiles
  (sliced shared-dst gathers serialize) on the 4 SWDGE queues.
- u side: zero per-edge descriptors. ge-staircase (one DVE is_ge per
  class vs a constant slot-iota tile, per-partition start scalars) +
  telescoping expansion matmul (rhs = host-side row differences).
- Compute emission is phase-split per tile so each engine streams:
  P1 expansion matmuls (PE) + row-space mults (DVE),
  P2 transposes (PE) + PSUM->SBUF copies (DVE/ACT),
  P3 wct matmuls (PE) + output copies (ACT).
"""

import numpy as np

import concourse.bacc as bacc
import concourse.mybir as mybir
import concourse.tile as tile
from concourse.bass_utils import run_bass_kernel_spmd

NUM_USERS = 100_000
NUM_ITEMS = 50_000
D = 64
C = 5
E = 2_000_000
NCORES = 8
P = 128

DX = 74
SEGW = 2048
QAS = 1024
QBS = SEGW - QAS
IPT = 2 * SEGW
VSPLIT = 25088
NVPAD = 50176
USEG_MAX = 128
NT0 = 63

f32 = mybir.dt.float32
fp16 = mybir.dt.float16
i16 = mybir.dt.int16


def build_nc(nt=NT0, reps=1):
    nc = bacc.Bacc(
        None, target_bir_lowering=False, debug=False, num_swdge_queues=4
    )
    vtab = nc.dram_tensor("vtab", [NVPAD, P], fp16, kind="ExternalInput")
    udiff = nc.dram_tensor("udiff", [nt, 2, P, P], fp16, kind="ExternalInput")
    starts = nc.dram_tensor("starts", [nt, P, 4], f32, kind="ExternalInput")
    vidx = nc.dram_tensor("vidx", [nt, P, IPT // 16], i16, kind="ExternalInput")
    wct = nc.dram_tensor("wct", [DX, C], fp16, kind="ExternalInput")
    iotas = nc.dram_tensor("iotas", [P, SEGW], fp16, kind="ExternalInput")
    iden = nc.dram_tensor("iden", [P, P], fp16, kind="ExternalInput")
    out = nc.dram_tensor("out", [nt, C, IPT], fp16, kind="ExternalOutput")

    vA = vtab[0:VSPLIT, :]
    vB = vtab[VSPLIT:NVPAD, :]
    G = IPT // P
    GA = QAS // P

    with tile.TileContext(nc) as tc:
        with (
            tc.tile_pool(name="const", bufs=1) as cpool,
            tc.tile_pool(name="gath", bufs=6) as gpool,
            tc.tile_pool(name="uwin", bufs=6) as wpool,
            tc.tile_pool(name="oh", bufs=5) as ohpool,
            tc.tile_pool(name="uvr", bufs=4) as uvrpool,
            tc.tile_pool(name="uvt", bufs=4) as uvtpool,
            tc.tile_pool(name="osb", bufs=4) as opool,
            tc.tile_pool(name="work", bufs=6) as pool,
            tc.tile_pool(name="pse", bufs=4, space="PSUM") as pepool,
            tc.tile_pool(name="pst", bufs=2, space="PSUM") as ptpool,
            tc.tile_pool(name="pso", bufs=2, space="PSUM") as popool,
        ):
            wct_sb = cpool.tile([DX, C], fp16)
            nc.sync.dma_start(out=wct_sb[:, :], in_=wct[:, :])
            iotas_sb = cpool.tile([P, SEGW], fp16)
            nc.sync.dma_start(out=iotas_sb[:, :], in_=iotas[:, :])
            iden_sb = cpool.tile([P, P], fp16)
            nc.sync.dma_start(out=iden_sb[:, :], in_=iden[:, :])

            loop = tc.For_i(0, reps)
            loop.__enter__()
            for t in range(nt):
                vix = pool.tile([P, IPT // 16], i16)
                nc.sync.dma_start(out=vix[:, :], in_=vidx[t, :, :])
                st = pool.tile([P, 4], f32)
                nc.sync.dma_start(out=st[:, :], in_=starts[t, :, :])

                vga, vgb, uwin = [], [], []
                for s in range(2):
                    ga = gpool.tile([P, GA, P], fp16, name=f"vga{s}")
                    nc.gpsimd.dma_gather(
                        ga[:, :, :], vA,
                        vix[:, (s * SEGW) // 16:(s * SEGW + QAS) // 16],
                        QAS, QAS, P,
                        transpose=False, single_packet=False,
                        queue_num=2 * s,
                    )
                    gb = gpool.tile([P, G // 2 - GA, P], fp16, name=f"vgb{s}")
                    nc.gpsimd.dma_gather(
                        gb[:, :, :], vB,
                        vix[:, (s * SEGW + QAS) // 16:((s + 1) * SEGW) // 16],
                        QBS, QBS, P,
                        transpose=False, single_packet=False,
                        queue_num=2 * s + 1,
                    )
                    vga.append(ga)
                    vgb.append(gb)
                    uw = wpool.tile([P, P], fp16, name=f"uw{s}")
                    nc.sync.dma_start(out=uw[:, :], in_=udiff[t, s, :, :])
                    uwin.append(uw)

                oh = []
                for s in range(2):
                    o = ohpool.tile([P, SEGW], fp16)
                    nc.vector.tensor_scalar(
                        out=o[:, 0:QAS],
                        in0=iotas_sb[:, 0:QAS],
                        scalar1=st[:, 2 * s:2 * s + 1],
                        scalar2=None,
                        op0=mybir.AluOpType.is_ge,
                    )
                    nc.vector.tensor_scalar(
                        out=o[:, QAS:SEGW],
                        in0=iotas_sb[:, QAS:SEGW],
                        scalar1=st[:, 2 * s + 1:2 * s + 2],
                        scalar2=None,
                        op0=mybir.AluOpType.is_ge,
                    )
                    oh.append(o)

                uvr = uvrpool.tile([P, G, DX], fp16, name="uvr")
                uvT = uvtpool.tile([DX, IPT], fp16, name="uvT")
                outsb = opool.tile([C, IPT], fp16)
                # P1: expansion matmuls stream on PE; mults chase on DVE
                for j in range(IPT // 512):
                    s = j // 4
                    sj = j % 4
                    pe4 = pepool.tile([P, 4, DX], f32)
                    for g in range(4):
                        nc.tensor.matmul(
                            pe4[:, g, :],
                            oh[s][:, (4 * sj + g) * P:(4 * sj + g + 1) * P],
                            uwin[s][:, 0:DX],
                            start=True,
                            stop=True,
                        )
                    lo, hi = 4 * sj, 4 * sj + 4
                    if hi <= GA:
                        src_ap = vga[s][:, lo:hi, 0:DX]
                    else:
                        src_ap = vgb[s][:, lo - GA:hi - GA, 0:DX]
                    nc.vector.tensor_mul(
                        out=uvr[:, 4 * j:4 * (j + 1), :],
                        in0=pe4[:, :, :],
                        in1=src_ap,
                    )
                # P2: transposes stream on PE; copies chase on DVE/ACT
                for jj in range(IPT // 1024):
                    pT = ptpool.tile([DX, 8, P], fp16)
                    for g in range(8):
                        nc.tensor.transpose(
                            out=pT[:, g, :],
                            in_=uvr[:, 8 * jj + g, :],
                            identity=iden_sb[:, :],
                        )
                    if jj % 2 == 0:
                        nc.vector.tensor_copy(
                            out=uvT[:, jj * 1024:(jj + 1) * 1024],
                            in_=pT[:, :, :],
                        )
                    else:
                        nc.scalar.activation(
                            out=uvT[:, jj * 1024:(jj + 1) * 1024],
                            in_=pT[:, :, :],
                            func=mybir.ActivationFunctionType.Copy,
                        )
                # P3: wct matmuls stream on PE; po copies chase on ACT
                for j in range(IPT // 512):
                    po = popool.tile([C, 512], f32)
                    nc.tensor.matmul(
                        po[:, :],
                        wct_sb[:, :],
                        uvT[:, j * 512:(j + 1) * 512],
                        start=True,
                        stop=True,
                    )
                    nc.scalar.activation(
                        out=outsb[:, j * 512:(j + 1) * 512], in_=po[:, :],
                        func=mybir.ActivationFunctionType.Copy,
                    )
                nc.sync.dma_start(out=out[t, :, :], in_=outsb[:, :])
            loop.__exit__(None, None, None)
    nc.compile()
    return nc


def _tables(v_feat, v_bias):
    vtab = np.zeros((NVPAD, P), dtype=np.float16)
    vtab[:NUM_ITEMS, :D] = v_feat
    vtab[:NUM_ITEMS, D:D + C] = 1.0
    vtab[:NUM_ITEMS, D + C:DX] = v_bias
    return vtab


def _urows(u_feat, u_bias):
    urows = np.zeros((NUM_USERS + P, P), dtype=np.float32)
    urows[:NUM_USERS, :D] = u_feat
    urows[:NUM_USERS, D:D + C] = u_bias
    urows[:NUM_USERS, D + C:DX] = 1.0
    return urows


def _wct(W, scalars):
    wc = (scalars.astype(np.float64).T @ W.astype(np.float64)).astype(np.float32)
    wct = np.zeros((DX, C), dtype=np.float16)
    wct[:D] = wc.T
    wct[D:D + C] = np.eye(C, dtype=np.float16)
    wct[D + C:] = np.eye(C, dtype=np.float16)
    return wct


def _pack_idx(vals16):
    nt = vals16.shape[0]
    a = vals16.reshape(nt, IPT // 16, 16).transpose(0, 2, 1)
    return np.ascontiguousarray(np.tile(a, (1, 8, 1)))


def _cut_segments(u, isB):
    n = len(u)
    ca0 = np.concatenate([[0], np.cumsum(~isB)])
    cb0 = np.concatenate([[0], np.cumsum(isB)])
    segs = []
    i = 0
    while i < n:
        jA = int(np.searchsorted(ca0, ca0[i] + QAS, side="right")) - 1
        jB = int(np.searchsorted(cb0, cb0[i] + QBS, side="right")) - 1
        jU = int(np.searchsorted(u, u[i] + USEG_MAX, side="left"))
        j = min(jA, jB, jU, n)
        assert j > i
        segs.append((i, j))
        i = j
    return segs


def plan_edges(u_idx, v_idx):
    order = np.argsort(u_idx, kind="stable")
    blk = E // NCORES
    max_nseg = 0
    core_data = []
    for m in range(NCORES):
        eids = order[m * blk:(m + 1) * blk]
        u = u_idx[eids]
        isB = v_idx[eids] >= VSPLIT
        segs = _cut_segments(u, isB)
        core_data.append((eids, u, isB, segs))
        max_nseg = max(max_nseg, len(segs))
    nt = (max_nseg + 1) // 2

    loc, stt, v16, ubs = [], [], [], []
    for m in range(NCORES):
        eids, u, isB, segs = core_data[m]
        ll = np.full((2 * nt, SEGW), -1, np.int64)
        vv = np.zeros((2 * nt, SEGW), np.int64)
        ub = np.zeros(2 * nt, np.int64)
        ss = np.full((2 * nt, 2, P), SEGW, np.float32)
        for s, (i, j) in enumerate(segs):
            e = eids[i:j]
            sB = isB[i:j]
            eA, eB = e[~sB], e[sB]
            ub[s] = u[i]
            ll[s, :len(eA)] = eA
            ll[s, QAS:QAS + len(eB)] = eB
            vv[s, :len(eA)] = v_idx[eA]
            vv[s, QAS:QAS + len(eB)] = v_idx[eB] - VSPLIT
            offA = u_idx[eA] - ub[s]
            offB = u_idx[eB] - ub[s]
            ss[s, 0, :] = np.searchsorted(offA, np.arange(P), side="left")
            ss[s, 1, :] = QAS + np.searchsorted(offB, np.arange(P), side="left")
        loc.append(ll.reshape(nt, IPT).ravel())
        sr = ss.reshape(nt, 2, 2, P)
        stt.append(np.ascontiguousarray(
            sr.transpose(0, 3, 1, 2).reshape(nt, P, 4)))
        v16.append(_pack_idx(vv.reshape(nt, IPT).astype(np.int16)))
        ubs.append(ub)
    return nt, loc, stt, v16, ubs


_CACHE = {}


def build_like(reps):
    return build_nc(_CACHE.get("nt", NT0), reps=reps)


def prepare(u_feat, v_feat, W, scalars, u_bias, v_bias, u_idx, v_idx):
    u_feat = np.asarray(u_feat, np.float32)
    v_feat = np.asarray(v_feat, np.float32)
    W = np.asarray(W, np.float32)
    scalars = np.asarray(scalars, np.float32)
    u_bias = np.asarray(u_bias, np.float32)
    v_bias = np.asarray(v_bias, np.float32)
    u_idx = np.asarray(u_idx, np.int32)
    v_idx = np.asarray(v_idx, np.int32)

    vtab = _tables(v_feat, v_bias)
    urows = _urows(u_feat, u_bias)
    wct = _wct(W, scalars)
    iotas = np.broadcast_to(
        np.arange(SEGW, dtype=np.float16)[None, :], (P, SEGW)
    ).copy()
    iden = np.eye(P, dtype=np.float16)
    nt, loc, stt, v16, ubs = plan_edges(u_idx, v_idx)

    if _CACHE.get("nt") != nt:
        _CACHE["nc"] = build_nc(nt)
        _CACHE["nt"] = nt
    nc = _CACHE["nc"]

    in_maps = []
    for m in range(NCORES):
        wins = urows[(ubs[m][:, None] + np.arange(P)[None, :]).reshape(-1)]
        wins = wins.reshape(2 * nt, P, P)
        diffs = np.empty_like(wins)
        diffs[:, 0, :] = wins[:, 0, :]
        diffs[:, 1:, :] = wins[:, 1:, :] - wins[:, :-1, :]
        in_maps.append(
            {"vtab": vtab,
             "udiff": np.ascontiguousarray(
                 diffs.astype(np.float16).reshape(nt, 2, P, P)),
             "starts": stt[m],
             "vidx": v16[m], "wct": wct, "iotas": iotas, "iden": iden}
        )
    return nc, in_maps, loc, nt


def assemble(core_outs, loc, nt):
    out_full = np.empty((E, C), dtype=np.float32)
    for m in range(NCORES):
        flat = np.asarray(core_outs[m]).transpose(0, 2, 1).reshape(nt * IPT, C)
        valid = loc[m] >= 0
        out_full[loc[m][valid]] = flat[valid]
    return out_full


def kernel(u_feat, v_feat, W, scalars, u_bias, v_bias, u_idx, v_idx):
    nc, in_maps, loc, nt = prepare(
        u_feat, v_feat, W, scalars, u_bias, v_bias, u_idx, v_idx
    )
    res = run_bass_kernel_spmd(nc, in_maps, core_ids=list(range(NCORES)))
    return assemble([r["out"] for r in res.results], loc, nt)


# revision 8
# speedup vs baseline: 1.3550x; 1.3550x over previous
"""Trainium2 Bass kernel v9 for nn_BilinearMixture.

Design (measurement-driven):
- Edges u-sorted, block-partitioned over 8 cores; cut into segments of
  <=2048 edges / <=128 distinct users with per-segment class quotas
  QAS=QBS=1024 (VSPLIT=25088 makes the v-table halves equiprobable, so
  all four per-tile dma_gather calls are EQUAL 1024-idx calls ->
  queue-parallel; unequal calls measured 2x slower).
- v side: 4 plain dma_gather calls/tile into SEPARATE dst tiles
  (sliced shared-dst gathers serialize) on the 4 SWDGE queues.
- u side: zero per-edge descriptors. ge-staircase (one DVE is_ge per
  class vs a constant slot-iota tile, per-partition start scalars) +
  telescoping expansion matmul (rhs = host-side row differences).
- Compute emission is phase-split per tile so each engine streams:
  P1 expansion matmuls (PE) + row-space mults (DVE),
  P2 transposes (PE) + PSUM->SBUF copies (DVE/ACT),
  P3 wct matmuls (PE) + output copies (ACT).
"""

import numpy as np

import concourse.bacc as bacc
import concourse.mybir as mybir
import concourse.tile as tile
from concourse.bass_utils import run_bass_kernel_spmd

NUM_USERS = 100_000
NUM_ITEMS = 50_000
D = 64
C = 5
E = 2_000_000
NCORES = 8
P = 128

DX = 74
SEGW = 2048
QAS = 1024
QBS = SEGW - QAS
IPT = 2 * SEGW
VSPLIT = 25088
NVPAD = 50176
USEG_MAX = 128
NT0 = 63

f32 = mybir.dt.float32
fp16 = mybir.dt.float16
i16 = mybir.dt.int16


def build_nc(nt=NT0, reps=1):
    nc = bacc.Bacc(
        None, target_bir_lowering=False, debug=False, num_swdge_queues=4
    )
    vtab = nc.dram_tensor("vtab", [NVPAD, P], fp16, kind="ExternalInput")
    udiff = nc.dram_tensor("udiff", [nt, 2, P, P], fp16, kind="ExternalInput")
    starts = nc.dram_tensor("starts", [nt, P, 4], f32, kind="ExternalInput")
    vidx = nc.dram_tensor("vidx", [nt, P, IPT // 16], i16, kind="ExternalInput")
    wct = nc.dram_tensor("wct", [DX, C], fp16, kind="ExternalInput")
    iotas = nc.dram_tensor("iotas", [P, SEGW], fp16, kind="ExternalInput")
    iden = nc.dram_tensor("iden", [P, P], fp16, kind="ExternalInput")
    out = nc.dram_tensor("out", [nt, C, IPT], fp16, kind="ExternalOutput")

    vA = vtab[0:VSPLIT, :]
    vB = vtab[VSPLIT:NVPAD, :]
    G = IPT // P
    GA = QAS // P

    with tile.TileContext(nc) as tc:
        with (
            tc.tile_pool(name="const", bufs=1) as cpool,
            tc.tile_pool(name="gath", bufs=6) as gpool,
            tc.tile_pool(name="uwin", bufs=6) as wpool,
            tc.tile_pool(name="oh", bufs=4) as ohpool,
            tc.tile_pool(name="uvr", bufs=3) as uvrpool,
            tc.tile_pool(name="uvt", bufs=4) as uvtpool,
            tc.tile_pool(name="osb", bufs=4) as opool,
            tc.tile_pool(name="work", bufs=6) as pool,
            tc.tile_pool(name="pse", bufs=4, space="PSUM") as pepool,
            tc.tile_pool(name="pst", bufs=2, space="PSUM") as ptpool,
            tc.tile_pool(name="pso", bufs=2, space="PSUM") as popool,
        ):
            wct_sb = cpool.tile([DX, C], fp16)
            nc.sync.dma_start(out=wct_sb[:, :], in_=wct[:, :])
            iotas_sb = cpool.tile([P, SEGW], fp16)
            nc.sync.dma_start(out=iotas_sb[:, :], in_=iotas[:, :])
            iden_sb = cpool.tile([P, P], fp16)
            nc.sync.dma_start(out=iden_sb[:, :], in_=iden[:, :])

            loop = tc.For_i(0, reps)
            loop.__enter__()
            for t in range(nt):
                vix = pool.tile([P, IPT // 16], i16)
                nc.sync.dma_start(out=vix[:, :], in_=vidx[t, :, :])
                st = pool.tile([P, 4], f32)
                nc.sync.dma_start(out=st[:, :], in_=starts[t, :, :])

                vga, vgb, uwin = [], [], []
                for s in range(2):
                    ga = gpool.tile([P, GA, P], fp16, name=f"vga{s}")
                    nc.gpsimd.dma_gather(
                        ga[:, :, :], vA,
                        vix[:, (s * SEGW) // 16:(s * SEGW + QAS) // 16],
                        QAS, QAS, P,
                        transpose=False, single_packet=False,
                        queue_num=2 * s,
                    )
                    gb = gpool.tile([P, G // 2 - GA, P], fp16, name=f"vgb{s}")
                    nc.gpsimd.dma_gather(
                        gb[:, :, :], vB,
                        vix[:, (s * SEGW + QAS) // 16:((s + 1) * SEGW) // 16],
                        QBS, QBS, P,
                        transpose=False, single_packet=False,
                        queue_num=2 * s + 1,
                    )
                    vga.append(ga)
                    vgb.append(gb)
                    uw = wpool.tile([P, P], fp16, name=f"uw{s}")
                    nc.sync.dma_start(out=uw[:, :], in_=udiff[t, s, :, :])
                    uwin.append(uw)

                oh = []
                for s in range(2):
                    o = ohpool.tile([P, SEGW], fp16)
                    nc.vector.tensor_scalar(
                        out=o[:, 0:QAS],
                        in0=iotas_sb[:, 0:QAS],
                        scalar1=st[:, 2 * s:2 * s + 1],
                        scalar2=None,
                        op0=mybir.AluOpType.is_ge,
                    )
                    nc.vector.tensor_scalar(
                        out=o[:, QAS:SEGW],
                        in0=iotas_sb[:, QAS:SEGW],
                        scalar1=st[:, 2 * s + 1:2 * s + 2],
                        scalar2=None,
                        op0=mybir.AluOpType.is_ge,
                    )
                    oh.append(o)

                uvr = uvrpool.tile([P, G, DX], fp16, name="uvr")
                uvT = uvtpool.tile([DX, IPT], fp16, name="uvT")
                outsb = opool.tile([C, IPT], fp16)
                # P1: expansion matmuls stream on PE; mults chase on DVE
                for j in range(IPT // 512):
                    s = j // 4
                    sj = j % 4
                    pe4 = pepool.tile([P, 4, DX], f32)
                    for g in range(4):
                        nc.tensor.matmul(
                            pe4[:, g, :],
                            oh[s][:, (4 * sj + g) * P:(4 * sj + g + 1) * P],
                            uwin[s][:, 0:DX],
                            start=True,
                            stop=True,
                        )
                    lo, hi = 4 * sj, 4 * sj + 4
                    if hi <= GA:
                        src_ap = vga[s][:, lo:hi, 0:DX]
                    else:
                        src_ap = vgb[s][:, lo - GA:hi - GA, 0:DX]
                    nc.vector.tensor_mul(
                        out=uvr[:, 4 * j:4 * (j + 1), :],
                        in0=pe4[:, :, :],
                        in1=src_ap,
                    )
                # P2: transposes stream on PE; copies chase on DVE/ACT
                for jj in range(IPT // 1024):
                    pT = ptpool.tile([DX, 8, P], fp16)
                    for g in range(8):
                        nc.tensor.transpose(
                            out=pT[:, g, :],
                            in_=uvr[:, 8 * jj + g, :],
                            identity=iden_sb[:, :],
                        )
                    if jj % 2 == 0:
                        nc.vector.tensor_copy(
                            out=uvT[:, jj * 1024:(jj + 1) * 1024],
                            in_=pT[:, :, :],
                        )
                    else:
                        nc.scalar.activation(
                            out=uvT[:, jj * 1024:(jj + 1) * 1024],
                            in_=pT[:, :, :],
                            func=mybir.ActivationFunctionType.Copy,
                        )
                # P3: wct matmuls stream on PE; po copies chase on ACT
                for j in range(IPT // 512):
                    po = popool.tile([C, 512], f32)
                    nc.tensor.matmul(
                        po[:, :],
                        wct_sb[:, :],
                        uvT[:, j * 512:(j + 1) * 512],
                        start=True,
                        stop=True,
                    )
                    nc.scalar.activation(
                        out=outsb[:, j * 512:(j + 1) * 512], in_=po[:, :],
                        func=mybir.ActivationFunctionType.Copy,
                    )
                nc.sync.dma_start(out=out[t, :, :], in_=outsb[:, :])
            loop.__exit__(None, None, None)
    nc.compile()
    return nc


def _tables(v_feat, v_bias):
    vtab = np.zeros((NVPAD, P), dtype=np.float16)
    vtab[:NUM_ITEMS, :D] = v_feat
    vtab[:NUM_ITEMS, D:D + C] = 1.0
    vtab[:NUM_ITEMS, D + C:DX] = v_bias
    return vtab


def _urows(u_feat, u_bias):
    urows = np.zeros((NUM_USERS + P, P), dtype=np.float32)
    urows[:NUM_USERS, :D] = u_feat
    urows[:NUM_USERS, D:D + C] = u_bias
    urows[:NUM_USERS, D + C:DX] = 1.0
    return urows


def _wct(W, scalars):
    wc = (scalars.astype(np.float64).T @ W.astype(np.float64)).astype(np.float32)
    wct = np.zeros((DX, C), dtype=np.float16)
    wct[:D] = wc.T
    wct[D:D + C] = np.eye(C, dtype=np.float16)
    wct[D + C:] = np.eye(C, dtype=np.float16)
    return wct


def _pack_idx(vals16):
    nt = vals16.shape[0]
    a = vals16.reshape(nt, IPT // 16, 16).transpose(0, 2, 1)
    return np.ascontiguousarray(np.tile(a, (1, 8, 1)))


def _cut_segments(u, isB):
    n = len(u)
    ca0 = np.concatenate([[0], np.cumsum(~isB)])
    cb0 = np.concatenate([[0], np.cumsum(isB)])
    segs = []
    i = 0
    while i < n:
        jA = int(np.searchsorted(ca0, ca0[i] + QAS, side="right")) - 1
        jB = int(np.searchsorted(cb0, cb0[i] + QBS, side="right")) - 1
        jU = int(np.searchsorted(u, u[i] + USEG_MAX, side="left"))
        j = min(jA, jB, jU, n)
        assert j > i
        segs.append((i, j))
        i = j
    return segs


def plan_edges(u_idx, v_idx):
    order = np.argsort(u_idx, kind="stable")
    blk = E // NCORES
    max_nseg = 0
    core_data = []
    for m in range(NCORES):
        eids = order[m * blk:(m + 1) * blk]
        u = u_idx[eids]
        isB = v_idx[eids] >= VSPLIT
        segs = _cut_segments(u, isB)
        core_data.append((eids, u, isB, segs))
        max_nseg = max(max_nseg, len(segs))
    nt = (max_nseg + 1) // 2

    loc, stt, v16, ubs = [], [], [], []
    for m in range(NCORES):
        eids, u, isB, segs = core_data[m]
        ll = np.full((2 * nt, SEGW), -1, np.int64)
        vv = np.zeros((2 * nt, SEGW), np.int64)
        ub = np.zeros(2 * nt, np.int64)
        ss = np.full((2 * nt, 2, P), SEGW, np.float32)
        for s, (i, j) in enumerate(segs):
            e = eids[i:j]
            sB = isB[i:j]
            eA, eB = e[~sB], e[sB]
            ub[s] = u[i]
            ll[s, :len(eA)] = eA
            ll[s, QAS:QAS + len(eB)] = eB
            vv[s, :len(eA)] = v_idx[eA]
            vv[s, QAS:QAS + len(eB)] = v_idx[eB] - VSPLIT
            offA = u_idx[eA] - ub[s]
            offB = u_idx[eB] - ub[s]
            ss[s, 0, :] = np.searchsorted(offA, np.arange(P), side="left")
            ss[s, 1, :] = QAS + np.searchsorted(offB, np.arange(P), side="left")
        loc.append(ll.reshape(nt, IPT).ravel())
        sr = ss.reshape(nt, 2, 2, P)
        stt.append(np.ascontiguousarray(
            sr.transpose(0, 3, 1, 2).reshape(nt, P, 4)))
        v16.append(_pack_idx(vv.reshape(nt, IPT).astype(np.int16)))
        ubs.append(ub)
    return nt, loc, stt, v16, ubs


_CACHE = {}


def build_like(reps):
    return build_nc(_CACHE.get("nt", NT0), reps=reps)


def prepare(u_feat, v_feat, W, scalars, u_bias, v_bias, u_idx, v_idx):
    u_feat = np.asarray(u_feat, np.float32)
    v_feat = np.asarray(v_feat, np.float32)
    W = np.asarray(W, np.float32)
    scalars = np.asarray(scalars, np.float32)
    u_bias = np.asarray(u_bias, np.float32)
    v_bias = np.asarray(v_bias, np.float32)
    u_idx = np.asarray(u_idx, np.int32)
    v_idx = np.asarray(v_idx, np.int32)

    vtab = _tables(v_feat, v_bias)
    urows = _urows(u_feat, u_bias)
    wct = _wct(W, scalars)
    iotas = np.broadcast_to(
        np.arange(SEGW, dtype=np.float16)[None, :], (P, SEGW)
    ).copy()
    iden = np.eye(P, dtype=np.float16)
    nt, loc, stt, v16, ubs = plan_edges(u_idx, v_idx)

    if _CACHE.get("nt") != nt:
        _CACHE["nc"] = build_nc(nt)
        _CACHE["nt"] = nt
    nc = _CACHE["nc"]

    in_maps = []
    for m in range(NCORES):
        wins = urows[(ubs[m][:, None] + np.arange(P)[None, :]).reshape(-1)]
        wins = wins.reshape(2 * nt, P, P)
        diffs = np.empty_like(wins)
        diffs[:, 0, :] = wins[:, 0, :]
        diffs[:, 1:, :] = wins[:, 1:, :] - wins[:, :-1, :]
        in_maps.append(
            {"vtab": vtab,
             "udiff": np.ascontiguousarray(
                 diffs.astype(np.float16).reshape(nt, 2, P, P)),
             "starts": stt[m],
             "vidx": v16[m], "wct": wct, "iotas": iotas, "iden": iden}
        )
    return nc, in_maps, loc, nt


def assemble(core_outs, loc, nt):
    out_full = np.empty((E, C), dtype=np.float32)
    for m in range(NCORES):
        flat = np.asarray(core_outs[m]).transpose(0, 2, 1).reshape(nt * IPT, C)
        valid = loc[m] >= 0
        out_full[loc[m][valid]] = flat[valid]
    return out_full


def kernel(u_feat, v_feat, W, scalars, u_bias, v_bias, u_idx, v_idx):
    nc, in_maps, loc, nt = prepare(
        u_feat, v_feat, W, scalars, u_bias, v_bias, u_idx, v_idx
    )
    res = run_bass_kernel_spmd(nc, in_maps, core_ids=list(range(NCORES)))
    return assemble([r["out"] for r in res.results], loc, nt)


# revision 9
# speedup vs baseline: 2.1191x; 1.5639x over previous
"""Trainium2 Bass kernel v9 for nn_BilinearMixture.

Design (measurement-driven):
- Edges u-sorted, block-partitioned over 8 cores; cut into segments of
  <=2048 edges / <=128 distinct users with per-segment class quotas
  QAS=QBS=1024 (VSPLIT=25088 makes the v-table halves equiprobable, so
  all four per-tile dma_gather calls are EQUAL 1024-idx calls ->
  queue-parallel; unequal calls measured 2x slower).
- v side: 4 plain dma_gather calls/tile into SEPARATE dst tiles
  (sliced shared-dst gathers serialize) on the 4 SWDGE queues.
- u side: zero per-edge descriptors. ge-staircase (one DVE is_ge per
  class vs a constant slot-iota tile, per-partition start scalars) +
  telescoping expansion matmul (rhs = host-side row differences).
- Compute emission is phase-split per tile so each engine streams:
  P1 expansion matmuls (PE) + row-space mults (DVE),
  P2 transposes (PE) + PSUM->SBUF copies (DVE/ACT),
  P3 wct matmuls (PE) + output copies (ACT).
"""

import numpy as np

import concourse.bacc as bacc
import concourse.mybir as mybir
import concourse.tile as tile
from concourse.bass_utils import run_bass_kernel_spmd

NUM_USERS = 100_000
NUM_ITEMS = 50_000
D = 64
C = 5
E = 2_000_000
NCORES = 8
P = 128

DX = 74
SEGW = 2048
QAS = 1024
QBS = SEGW - QAS
IPT = 2 * SEGW
VSPLIT = 25088
NVPAD = 50176
USEG_MAX = 128
NT0 = 63

f32 = mybir.dt.float32
fp16 = mybir.dt.float16
i16 = mybir.dt.int16


def build_nc(nt=NT0, reps=1):
    nc = bacc.Bacc(
        None, target_bir_lowering=False, debug=False, num_swdge_queues=4
    )
    vtab = nc.dram_tensor("vtab", [NVPAD, P], fp16, kind="ExternalInput")
    udiff = nc.dram_tensor("udiff", [nt, 2, P, P], fp16, kind="ExternalInput")
    starts = nc.dram_tensor("starts", [nt, P, 4], f32, kind="ExternalInput")
    vidx = nc.dram_tensor("vidx", [nt, P, IPT // 16], i16, kind="ExternalInput")
    wct = nc.dram_tensor("wct", [DX, C], fp16, kind="ExternalInput")
    iotas = nc.dram_tensor("iotas", [P, SEGW], fp16, kind="ExternalInput")
    iden = nc.dram_tensor("iden", [P, P], fp16, kind="ExternalInput")
    out = nc.dram_tensor("out", [nt, C, IPT], fp16, kind="ExternalOutput")

    vA = vtab[0:VSPLIT, :]
    vB = vtab[VSPLIT:NVPAD, :]
    G = IPT // P
    GA = QAS // P

    with tile.TileContext(nc) as tc:
        with (
            tc.tile_pool(name="const", bufs=1) as cpool,
            tc.tile_pool(name="gath", bufs=7) as gpool,
            tc.tile_pool(name="uwin", bufs=6) as wpool,
            tc.tile_pool(name="oh", bufs=4) as ohpool,
            tc.tile_pool(name="uvr", bufs=3) as uvrpool,
            tc.tile_pool(name="uvt", bufs=4) as uvtpool,
            tc.tile_pool(name="osb", bufs=4) as opool,
            tc.tile_pool(name="work", bufs=6) as pool,
            tc.tile_pool(name="pse", bufs=4, space="PSUM") as pepool,
            tc.tile_pool(name="pst", bufs=2, space="PSUM") as ptpool,
            tc.tile_pool(name="pso", bufs=2, space="PSUM") as popool,
        ):
            wct_sb = cpool.tile([DX, C], fp16)
            nc.sync.dma_start(out=wct_sb[:, :], in_=wct[:, :])
            iotas_sb = cpool.tile([P, SEGW], fp16)
            nc.sync.dma_start(out=iotas_sb[:, :], in_=iotas[:, :])
            iden_sb = cpool.tile([P, P], fp16)
            nc.sync.dma_start(out=iden_sb[:, :], in_=iden[:, :])

            loop = tc.For_i(0, reps)
            loop.__enter__()
            for t in range(nt):
                vix = pool.tile([P, IPT // 16], i16)
                nc.sync.dma_start(out=vix[:, :], in_=vidx[t, :, :])
                st = pool.tile([P, 4], f32)
                nc.sync.dma_start(out=st[:, :], in_=starts[t, :, :])

                vga, vgb, uwin = [], [], []
                for s in range(2):
                    ga = gpool.tile([P, GA, P], fp16, name=f"vga{s}")
                    nc.gpsimd.dma_gather(
                        ga[:, :, :], vA,
                        vix[:, (s * SEGW) // 16:(s * SEGW + QAS) // 16],
                        QAS, QAS, P,
                        transpose=False, single_packet=False,
                        queue_num=2 * s,
                    )
                    gb = gpool.tile([P, G // 2 - GA, P], fp16, name=f"vgb{s}")
                    nc.gpsimd.dma_gather(
                        gb[:, :, :], vB,
                        vix[:, (s * SEGW + QAS) // 16:((s + 1) * SEGW) // 16],
                        QBS, QBS, P,
                        transpose=False, single_packet=False,
                        queue_num=2 * s + 1,
                    )
                    vga.append(ga)
                    vgb.append(gb)
                    uw = wpool.tile([P, P], fp16, name=f"uw{s}")
                    nc.sync.dma_start(out=uw[:, :], in_=udiff[t, s, :, :])
                    uwin.append(uw)

                oh = []
                for s in range(2):
                    o = ohpool.tile([P, SEGW], fp16)
                    nc.vector.tensor_scalar(
                        out=o[:, 0:QAS],
                        in0=iotas_sb[:, 0:QAS],
                        scalar1=st[:, 2 * s:2 * s + 1],
                        scalar2=None,
                        op0=mybir.AluOpType.is_ge,
                    )
                    nc.vector.tensor_scalar(
                        out=o[:, QAS:SEGW],
                        in0=iotas_sb[:, QAS:SEGW],
                        scalar1=st[:, 2 * s + 1:2 * s + 2],
                        scalar2=None,
                        op0=mybir.AluOpType.is_ge,
                    )
                    oh.append(o)

                uvr = uvrpool.tile([P, G, DX], fp16, name="uvr")
                uvT = uvtpool.tile([DX, IPT], fp16, name="uvT")
                outsb = opool.tile([C, IPT], fp16)
                # P1: expansion matmuls stream on PE; mults chase on DVE
                for j in range(IPT // 512):
                    s = j // 4
                    sj = j % 4
                    pe4 = pepool.tile([P, 4, DX], f32)
                    for g in range(4):
                        nc.tensor.matmul(
                            pe4[:, g, :],
                            oh[s][:, (4 * sj + g) * P:(4 * sj + g + 1) * P],
                            uwin[s][:, 0:DX],
                            start=True,
                            stop=True,
                        )
                    lo, hi = 4 * sj, 4 * sj + 4
                    if hi <= GA:
                        src_ap = vga[s][:, lo:hi, 0:DX]
                    else:
                        src_ap = vgb[s][:, lo - GA:hi - GA, 0:DX]
                    nc.vector.tensor_mul(
                        out=uvr[:, 4 * j:4 * (j + 1), :],
                        in0=pe4[:, :, :],
                        in1=src_ap,
                    )
                # P2: transposes stream on PE; copies chase on DVE/ACT
                for jj in range(IPT // 1024):
                    pT = ptpool.tile([DX, 8, P], fp16)
                    for g in range(8):
                        nc.tensor.transpose(
                            out=pT[:, g, :],
                            in_=uvr[:, 8 * jj + g, :],
                            identity=iden_sb[:, :],
                        )
                    if jj % 2 == 0:
                        nc.vector.tensor_copy(
                            out=uvT[:, jj * 1024:(jj + 1) * 1024],
                            in_=pT[:, :, :],
                        )
                    else:
                        nc.scalar.activation(
                            out=uvT[:, jj * 1024:(jj + 1) * 1024],
                            in_=pT[:, :, :],
                            func=mybir.ActivationFunctionType.Copy,
                        )
                # P3: wct matmuls stream on PE; po copies chase on ACT
                for j in range(IPT // 512):
                    po = popool.tile([C, 512], f32)
                    nc.tensor.matmul(
                        po[:, :],
                        wct_sb[:, :],
                        uvT[:, j * 512:(j + 1) * 512],
                        start=True,
                        stop=True,
                    )
                    nc.scalar.activation(
                        out=outsb[:, j * 512:(j + 1) * 512], in_=po[:, :],
                        func=mybir.ActivationFunctionType.Copy,
                    )
                nc.sync.dma_start(out=out[t, :, :], in_=outsb[:, :])
            loop.__exit__(None, None, None)
    nc.compile()
    return nc


def _tables(v_feat, v_bias):
    vtab = np.zeros((NVPAD, P), dtype=np.float16)
    vtab[:NUM_ITEMS, :D] = v_feat
    vtab[:NUM_ITEMS, D:D + C] = 1.0
    vtab[:NUM_ITEMS, D + C:DX] = v_bias
    return vtab


def _urows(u_feat, u_bias):
    urows = np.zeros((NUM_USERS + P, P), dtype=np.float32)
    urows[:NUM_USERS, :D] = u_feat
    urows[:NUM_USERS, D:D + C] = u_bias
    urows[:NUM_USERS, D + C:DX] = 1.0
    return urows


def _wct(W, scalars):
    wc = (scalars.astype(np.float64).T @ W.astype(np.float64)).astype(np.float32)
    wct = np.zeros((DX, C), dtype=np.float16)
    wct[:D] = wc.T
    wct[D:D + C] = np.eye(C, dtype=np.float16)
    wct[D + C:] = np.eye(C, dtype=np.float16)
    return wct


def _pack_idx(vals16):
    nt = vals16.shape[0]
    a = vals16.reshape(nt, IPT // 16, 16).transpose(0, 2, 1)
    return np.ascontiguousarray(np.tile(a, (1, 8, 1)))


def _cut_segments(u, isB):
    n = len(u)
    ca0 = np.concatenate([[0], np.cumsum(~isB)])
    cb0 = np.concatenate([[0], np.cumsum(isB)])
    segs = []
    i = 0
    while i < n:
        jA = int(np.searchsorted(ca0, ca0[i] + QAS, side="right")) - 1
        jB = int(np.searchsorted(cb0, cb0[i] + QBS, side="right")) - 1
        jU = int(np.searchsorted(u, u[i] + USEG_MAX, side="left"))
        j = min(jA, jB, jU, n)
        assert j > i
        segs.append((i, j))
        i = j
    return segs


def plan_edges(u_idx, v_idx):
    order = np.argsort(u_idx, kind="stable")
    blk = E // NCORES
    max_nseg = 0
    core_data = []
    for m in range(NCORES):
        eids = order[m * blk:(m + 1) * blk]
        u = u_idx[eids]
        isB = v_idx[eids] >= VSPLIT
        segs = _cut_segments(u, isB)
        core_data.append((eids, u, isB, segs))
        max_nseg = max(max_nseg, len(segs))
    nt = (max_nseg + 1) // 2

    loc, stt, v16, ubs = [], [], [], []
    for m in range(NCORES):
        eids, u, isB, segs = core_data[m]
        ll = np.full((2 * nt, SEGW), -1, np.int64)
        vv = np.zeros((2 * nt, SEGW), np.int64)
        ub = np.zeros(2 * nt, np.int64)
        ss = np.full((2 * nt, 2, P), SEGW, np.float32)
        for s, (i, j) in enumerate(segs):
            e = eids[i:j]
            sB = isB[i:j]
            eA, eB = e[~sB], e[sB]
            ub[s] = u[i]
            ll[s, :len(eA)] = eA
            ll[s, QAS:QAS + len(eB)] = eB
            vv[s, :len(eA)] = v_idx[eA]
            vv[s, QAS:QAS + len(eB)] = v_idx[eB] - VSPLIT
            offA = u_idx[eA] - ub[s]
            offB = u_idx[eB] - ub[s]
            ss[s, 0, :] = np.searchsorted(offA, np.arange(P), side="left")
            ss[s, 1, :] = QAS + np.searchsorted(offB, np.arange(P), side="left")
        loc.append(ll.reshape(nt, IPT).ravel())
        sr = ss.reshape(nt, 2, 2, P)
        stt.append(np.ascontiguousarray(
            sr.transpose(0, 3, 1, 2).reshape(nt, P, 4)))
        v16.append(_pack_idx(vv.reshape(nt, IPT).astype(np.int16)))
        ubs.append(ub)
    return nt, loc, stt, v16, ubs


_CACHE = {}


def build_like(reps):
    return build_nc(_CACHE.get("nt", NT0), reps=reps)


def prepare(u_feat, v_feat, W, scalars, u_bias, v_bias, u_idx, v_idx):
    u_feat = np.asarray(u_feat, np.float32)
    v_feat = np.asarray(v_feat, np.float32)
    W = np.asarray(W, np.float32)
    scalars = np.asarray(scalars, np.float32)
    u_bias = np.asarray(u_bias, np.float32)
    v_bias = np.asarray(v_bias, np.float32)
    u_idx = np.asarray(u_idx, np.int32)
    v_idx = np.asarray(v_idx, np.int32)

    vtab = _tables(v_feat, v_bias)
    urows = _urows(u_feat, u_bias)
    wct = _wct(W, scalars)
    iotas = np.broadcast_to(
        np.arange(SEGW, dtype=np.float16)[None, :], (P, SEGW)
    ).copy()
    iden = np.eye(P, dtype=np.float16)
    nt, loc, stt, v16, ubs = plan_edges(u_idx, v_idx)

    if _CACHE.get("nt") != nt:
        _CACHE["nc"] = build_nc(nt)
        _CACHE["nt"] = nt
    nc = _CACHE["nc"]

    in_maps = []
    for m in range(NCORES):
        wins = urows[(ubs[m][:, None] + np.arange(P)[None, :]).reshape(-1)]
        wins = wins.reshape(2 * nt, P, P)
        diffs = np.empty_like(wins)
        diffs[:, 0, :] = wins[:, 0, :]
        diffs[:, 1:, :] = wins[:, 1:, :] - wins[:, :-1, :]
        in_maps.append(
            {"vtab": vtab,
             "udiff": np.ascontiguousarray(
                 diffs.astype(np.float16).reshape(nt, 2, P, P)),
             "starts": stt[m],
             "vidx": v16[m], "wct": wct, "iotas": iotas, "iden": iden}
        )
    return nc, in_maps, loc, nt


def assemble(core_outs, loc, nt):
    out_full = np.empty((E, C), dtype=np.float32)
    for m in range(NCORES):
        flat = np.asarray(core_outs[m]).transpose(0, 2, 1).reshape(nt * IPT, C)
        valid = loc[m] >= 0
        out_full[loc[m][valid]] = flat[valid]
    return out_full


def kernel(u_feat, v_feat, W, scalars, u_bias, v_bias, u_idx, v_idx):
    nc, in_maps, loc, nt = prepare(
        u_feat, v_feat, W, scalars, u_bias, v_bias, u_idx, v_idx
    )
    res = run_bass_kernel_spmd(nc, in_maps, core_ids=list(range(NCORES)))
    return assemble([r["out"] for r in res.results], loc, nt)
